# revision 30
# baseline (speedup 1.0000x reference)
"""Trainium2 Bass kernel for nn_LocalDiscriminator (patch-GAN style loss).

Reference computation (full shapes):
    x: [32, 1024, 64, 64] f32, w: [1, 1024] f32, b: [1] f32, mode: scalar int
    logits = einsum('bchw,c->bhw', x, w[0]) + b[0]
    z = sigmoid(logits)
    loss = mean(softplus(z) - z * mode)        # scalar f32

Strategy: data-parallel over the batch dim — 4 batches per core on 8 cores.
Each core streams its 64 MiB shard of x through the TensorEngine: the
channel contraction uses lhsT = [w_col, w_col] ([128, 2] stationary, f32r
so the moving data streams at 1 cycle/row instead of fp32's 4), writing
IDENTICAL logits to two PSUM partitions. One ScalarEngine tanh per
[2, 2048] group — with per-partition scale/bias APs — then evaluates both
reductions at once, and its accum_out port emits the per-group sums for
free (one ACT LUT table for the whole kernel; VectorE does one final
reduction only):
    partition 0:  sum tanh(FA*t + FA*b+FB)   -> softplus fit
    partition 1:  sum tanh(0.5*t + 0.5*b)    -> exact sigmoid identity
where t is the raw logit. Host combination:
    sum(z)            = N/2 + S_z/2                             (exact)
    sum(softplus(z)) ~= N*FC0 + FC1*S_f                         (fitted)
    loss = (sum(softplus(z)) - mode*sum(z)) / N
The fit softplus(sigmoid(t)) ~= FC0 + FC1*tanh(FA*t+FB) has max |err|
9.8e-4 per element on t in [-4.5, 4.5] and its mean error over the
N(0, ~0.64) logit distribution cancels to ~1e-7 — the loss is a mean of
131072 such elements, so even the worst-case systematic error (1.4e-3,
saturated logits) is 14x inside the 2e-2 gate.
The kernel is HBM-bandwidth-bound: ~64 MiB/core at ~358 GB/s (~187 us);
the TimelineSim cost model predicts 203 us/core end-to-end.
"""

import os
import sys

import numpy as np

_REPO_CANDIDATES = ("/opt/trn_rl_repo", "/root/.axon_site/_ro/trn_rl_repo")
for _p in _REPO_CANDIDATES:
    if os.path.isdir(_p) and _p not in sys.path:
        sys.path.insert(0, _p)

import concourse.bacc as bacc
import concourse.bass as bass
import concourse.mybir as mybir
import concourse.tile as tile
from concourse.bass_utils import run_bass_kernel_spmd

N_CORES = 8
B_FULL, C, H, W = 32, 1024, 64, 64
B_LOCAL = B_FULL // N_CORES          # 4 batches per core
HW = H * W                           # 4096 spatial positions per batch
C_CHUNKS = C // 128                  # 8 chunks of 128 channels
BANKS = 4                            # psum banks per group: [1, 4, 512]
GRP = BANKS * 512                    # 2048 positions per pointwise group
GRPS = HW // GRP                     # 2 groups per batch
N_LOCAL = B_LOCAL * HW               # positions per core

# softplus(sigmoid(t)) ~= FC0 + FC1 * tanh(FA*t + FB)
FC0 = 1.0028824947566075
FC1 = 0.30899789558232016
FA = 0.5078652298016119
FB = -0.09351045988102749

F32 = mybir.dt.float32
F32R = mybir.dt.float32r

_nc_cache = None
_exec_cache = None


def _build_nc():
    nc = bacc.Bacc("TRN2", target_bir_lowering=False, debug=False,
                   num_devices=N_CORES)

    x = nc.dram_tensor("x", [B_LOCAL, C, H, W], F32, kind="ExternalInput").ap()
    w = nc.dram_tensor("w", [1, C], F32, kind="ExternalInput").ap()
    # aff[p] = (scale, bias) for the tanh on psum partition p; computed on
    # the host from the Linear bias b:
    #   row 0 = (FA, FA*b+FB)   (softplus fit), row 1 = (0.5, 0.5*b) (sigmoid)
    aff = nc.dram_tensor("aff", [2, 2], F32, kind="ExternalInput").ap()
    out = nc.dram_tensor("out", [2], F32, kind="ExternalOutput").ap()

    xr = x.rearrange("b c h w -> b c (h w)")  # [B_LOCAL, 1024, 4096]

    with tile.TileContext(nc) as tc:
        with (
            tc.tile_pool(name="xpool", bufs=5) as xpool,
            tc.tile_pool(name="const", bufs=1) as cpool,
            tc.tile_pool(name="sums", bufs=1) as spool,
            tc.tile_pool(name="dump", bufs=2) as dpool,
            tc.tile_pool(name="psum", bufs=2, space="PSUM") as pspool,
        ):
            # Two copies of w side by side: lhsT [128, 2] per chunk makes the
            # matmul write identical logits to TWO psum partitions, so one
            # ACT tanh with per-partition scale/bias evaluates both the
            # softplus fit (partition 0) and the exact sigmoid identity
            # (partition 1) in a single instruction.
            # w2[p, k, j] = w[0, j*128 + p] for k in {0,1}.
            w2 = cpool.tile([128, 2, C_CHUNKS], F32R, tag="w")
            for k in range(2):
                nc.gpsimd.dma_start(
                    out=w2[:, k, :],
                    in_=w[0].bitcast(F32R).rearrange("(j p) -> p j", p=128))
            aff_t = cpool.tile([2, 2], F32, tag="aff")
            nc.gpsimd.dma_start(out=aff_t[:], in_=aff[:])

            # sums[0, i] = sum tanh(FA*t+FB') of group i  (softplus fit)
            # sums[1, i] = sum tanh(t/2+b/2) of group i   (sigmoid)
            sums = spool.tile([2, B_LOCAL * GRPS], F32, tag="sums")

            for bi in range(B_LOCAL):
                # Load channel-chunk PAIRS: one contiguous 4 MiB block per
                # dma_start ([128, 2*HW] tile; chunk c at columns
                # (c%2)*HW : (c%2+1)*HW).  4 MiB transfers clear the ~91%
                # DMA-efficiency knee, so the stream runs at the HBM ceiling.
                # The last batch's final pair is split in two so the tail
                # dependency (chunk 7) lands sooner.
                last = bi == B_LOCAL - 1
                chunk_slices = {}
                for p in range(C_CHUNKS // 2):
                    xt = xpool.tile([128, 2 * HW], F32R, tag="x")
                    if last and p == C_CHUNKS // 2 - 1:
                        # two half-DMAs into one tile: chunk-6 matmuls can
                        # start before chunk 7 arrives
                        for c in (2 * p, 2 * p + 1):
                            nc.sync.dma_start(
                                out=xt[:, (c % 2) * HW:(c % 2 + 1) * HW],
                                in_=xr[bi, bass.ts(c, 128), :].bitcast(F32R))
                    else:
                        nc.sync.dma_start(
                            out=xt[:],
                            in_=xr[bi, 256 * p:256 * (p + 1), :].bitcast(F32R))
                    chunk_slices[2 * p] = (xt, 0)
                    chunk_slices[2 * p + 1] = (xt, HW)
                for g in range(GRPS):
                    idx = bi * GRPS + g
                    ps = pspool.tile([2, BANKS, 512], F32, tag="ps")
                    for jj in range(BANKS):
                        col0 = g * GRP + jj * 512
                        for c in range(C_CHUNKS):
                            xt, off = chunk_slices[c]
                            nc.tensor.matmul(
                                ps[0:2, jj, :],
                                lhsT=w2[:, :, c],
                                rhs=xt[:, off + col0:off + col0 + 512],
                                start=(c == 0),
                                stop=(c == C_CHUNKS - 1),
                            )
                    # Only the accum_out sums are consumed; the elementwise
                    # tanh output goes to a scratch tile.
                    dump = dpool.tile([2, GRP], F32, tag="dump")
                    nc.scalar.activation(
                        dump[:], ps[0:2].rearrange("p a b -> p (a b)"),
                        mybir.ActivationFunctionType.Tanh,
                        bias=aff_t[:, 1:2], scale=aff_t[:, 0:1],
                        accum_out=sums[0:2, idx:idx + 1],
                    )

            fin = spool.tile([2, 1], F32, tag="fin")
            nc.vector.reduce_sum(out=fin[:], in_=sums[:],
                                 axis=mybir.AxisListType.X)
            nc.sync.dma_start(out=out[:, None], in_=fin[:])

    nc.compile()
    return nc


def _get_nc():
    global _nc_cache
    if _nc_cache is None:
        _nc_cache = _build_nc()
    return _nc_cache


def _get_exec():
    """Compile the 8-core SPMD executable once and cache the jitted callable
    (run_bass_kernel_spmd rebuilds + recompiles the NEFF on every call)."""
    global _exec_cache
    if _exec_cache is not None:
        return _exec_cache

    import jax
    import concourse.mybir as _mybir
    from concourse import bass2jax
    from jax.experimental.shard_map import shard_map
    from jax.sharding import Mesh, PartitionSpec

    nc = _get_nc()
    bass2jax.install_neuronx_cc_hook()

    partition_name = (nc.partition_id_tensor.name
                      if nc.partition_id_tensor else None)
    in_names, out_names, out_avals = [], [], []
    for alloc in nc.m.functions[0].allocations:
        if not isinstance(alloc, _mybir.MemoryLocationSet):
            continue
        name = alloc.memorylocations[0].name
        if alloc.kind == "ExternalInput":
            if name != partition_name:
                in_names.append(name)
        elif alloc.kind == "ExternalOutput":
            shape = tuple(alloc.tensor_shape)
            dtype = _mybir.dt.np(alloc.dtype)
            out_names.append(name)
            out_avals.append(jax.core.ShapedArray(shape, dtype))
    n_params = len(in_names)
    all_in_names = list(in_names) + list(out_names)
    if partition_name is not None:
        all_in_names.append(partition_name)

    def _body(*args):
        operands = list(args)
        if partition_name is not None:
            operands.append(bass2jax.partition_id_tensor())
        outs = bass2jax._bass_exec_p.bind(
            *operands,
            out_avals=tuple(out_avals),
            in_names=tuple(all_in_names),
            out_names=tuple(out_names),
            lowering_input_output_aliases=(),
            sim_require_finite=True,
            sim_require_nnan=True,
            nc=nc,
        )
        return tuple(outs)

    devices = jax.devices()[:N_CORES]
    mesh = Mesh(np.asarray(devices), ("core",))
    n_outs = len(out_names)
    sharded = jax.jit(
        shard_map(
            _body, mesh=mesh,
            in_specs=(PartitionSpec("core"),) * (n_params + n_outs),
            out_specs=(PartitionSpec("core"),) * n_outs,
            check_rep=False,
        ),
        donate_argnums=tuple(range(n_params, n_params + n_outs)),
        keep_unused=True,
    )
    _exec_cache = (sharded, in_names, out_names, out_avals)
    return _exec_cache


def _run_spmd(in_maps):
    """Run the cached executable; returns list of per-core output dicts."""
    sharded, in_names, out_names, out_avals = _get_exec()
    concat_in = [
        np.concatenate([np.asarray(m[name]) for m in in_maps], axis=0)
        for name in in_names
    ]
    concat_zeros = [
        np.zeros((N_CORES * av.shape[0], *av.shape[1:]), av.dtype)
        for av in out_avals
    ]
    out_arrs = sharded(*concat_in, *concat_zeros)
    return [
        {name: np.asarray(out_arrs[i]).reshape(N_CORES, *out_avals[i].shape)[c]
         for i, name in enumerate(out_names)}
        for c in range(N_CORES)
    ]


def kernel(x: np.ndarray, w: np.ndarray, b: np.ndarray, mode) -> np.ndarray:
    x = np.ascontiguousarray(np.asarray(x, dtype=np.float32))
    w = np.ascontiguousarray(np.asarray(w, dtype=np.float32))
    b = np.ascontiguousarray(np.asarray(b, dtype=np.float32))
    assert x.shape == (B_FULL, C, H, W), x.shape

    b0 = float(b.reshape(-1)[0])
    aff = np.array([[FA, FA * b0 + FB], [0.5, 0.5 * b0]], dtype=np.float32)
    in_maps = [
        {"x": x[i * B_LOCAL:(i + 1) * B_LOCAL], "w": w, "aff": aff}
        for i in range(N_CORES)
    ]
    try:
        results = _run_spmd(in_maps)
    except Exception:
        nc = _get_nc()
        results = run_bass_kernel_spmd(nc, in_maps, list(range(N_CORES))).results
    partial = np.stack([r["out"] for r in results])  # [8, (sum_f, sum_z)]

    n_total = float(B_FULL * HW)
    sum_f = float(partial[:, 0].sum())
    sum_z = float(partial[:, 1].sum())
    s_sp = n_total * FC0 + FC1 * sum_f
    s_z = n_total / 2.0 + sum_z / 2.0
    y = float(np.asarray(mode))
    loss = (s_sp - y * s_z) / n_total
    return np.float32(loss)
